# revision 2
# baseline (speedup 1.0000x reference)
"""DeepSeek V3.1 MLA attention (B=1, S=2048, D=4096, H=32) on 8 TRN2 NeuronCores.

Sharding: tensor-parallel across heads (4 heads/core) for the q-up /
attention / output path, token-parallel for the shared projections.
The replicated low-rank projections of the baseline (c_q via a folded
W_q weight, c_kv/k_rope) are instead computed on 1/8 of the tokens per
core and exchanged with two DRAM AllGather collectives, cutting the
per-core PE work for those phases ~6x.

Attention runs "transposed": scores^T [t, q] tiles are produced directly
by PE matmuls from the resident ckv^T latents (stationary) and the
staged q-latents (moving), exp'd on the scalar engine into probsT, and
contracted against token-major ckv tiles to accumulate out_lat^T in
PSUM - no per-tile PE transposes of probabilities. The softmax
denominator rides a ones-column matmul accumulated alongside; the
normalization happens once per (head, 512-token q-group) on the final
per-head output tile, using a gpsimd partition-broadcast of 1/denom.

Matmul operands are fp16 (fp32 PSUM accumulation): halves DMA / SBUF /
collective bytes vs f32r and cuts LDWEIGHTS in half, at ~8x more
mantissa than bf16 (end-to-end rel-err ~8e-4).
Loads issue on the sync queue, stores on the gpsimd queue (ahead of the
collectives they feed) so stores never block downstream loads.
"""

import math
from contextlib import ExitStack
from dataclasses import dataclass

import numpy as np

import concourse.bass as bass
import concourse.bacc as bacc
import concourse.mybir as mybir
import concourse.tile as tile
from concourse.bass_utils import run_bass_kernel_spmd

F32 = mybir.dt.float32
F16 = mybir.dt.float16
EXP = mybir.ActivationFunctionType.Exp
AX = mybir.AxisListType.X
MASK_NEG = -1.0e30

# rope constants (must match the reference)
BASE = 10000.0
FACTOR = 40.0
BFAST, BSLOW = 32.0, 1.0
OLD_CTX = 4096.0
MSCALE = 1.0


@dataclass(frozen=True)
class Cfg:
    S: int = 2048
    D: int = 4096
    QL: int = 1536
    KVL: int = 512
    DN: int = 128
    DR: int = 64
    DV: int = 128
    H: int = 32
    n_cores: int = 8

    @property
    def HC(self):  # heads per core
        return self.H // self.n_cores

    @property
    def QH(self):  # per-core q-up output cols (nope then rope)
        return self.HC * self.DN + self.HC * self.DR

    @property
    def DC(self):  # d (model dim) 128-chunks
        return self.D // 128

    @property
    def QLC(self):
        return self.QL // 128

    @property
    def KC(self):
        return self.KVL // 128

    @property
    def SG(self):  # 512-token groups
        return self.S // 512

    @property
    def NT(self):  # 128-token chunks
        return self.S // 128

    @property
    def TOK(self):  # tokens per core (shard)
        return self.S // self.n_cores

    @property
    def TC(self):  # 128-token chunks per shard
        return self.TOK // 128

    @property
    def HDR(self):
        return self.DR // 2

    @property
    def scale(self):
        return 1.0 / math.sqrt(self.DN + self.DR)


def build_bass(cfg: Cfg, repeat: int = 1):
    """Build + compile the per-core SPMD bass program."""
    nc = bacc.Bacc("TRN2", target_bir_lowering=False, debug=False)
    S, D, QL, KVL, DN, DR, DV = (
        cfg.S, cfg.D, cfg.QL, cfg.KVL, cfg.DN, cfg.DR, cfg.DV,
    )
    HC, QH, DC, QLC, KC, SG, NT, HDR = (
        cfg.HC, cfg.QH, cfg.DC, cfg.QLC, cfg.KC, cfg.SG, cfg.NT, cfg.HDR,
    )
    TOK, TC, NCORE = cfg.TOK, cfg.TC, cfg.n_cores
    KR = KVL + DR  # 576
    J3 = QL // 512  # c_q 512-col groups

    # ---- kernel I/O ----
    hTs = nc.dram_tensor("hTs", [D, TOK], F16, kind="ExternalInput")
    wqd = nc.dram_tensor("wqd", [D, QL], F16, kind="ExternalInput")
    wqu = nc.dram_tensor("wqu", [QL, QH], F16, kind="ExternalInput")
    wkvr = nc.dram_tensor("wkvr", [D, KR], F16, kind="ExternalInput")
    wuk = nc.dram_tensor("wuk", [HC * DN, KVL], F16, kind="ExternalInput")
    wuvT = nc.dram_tensor("wuvT", [KVL, HC * DV], F16, kind="ExternalInput")
    wout = nc.dram_tensor("wout", [HC * DV, D], F16, kind="ExternalInput")
    ropeq = nc.dram_tensor("ropeq", [DR, S], F32, kind="ExternalInput")
    ropeks = nc.dram_tensor("ropeks", [DR, TOK], F32, kind="ExternalInput")
    trimask = nc.dram_tensor("trimask", [128, 128], F32, kind="ExternalInput")
    identD = nc.dram_tensor("identD", [128, 128], F16, kind="ExternalInput")
    partialT = nc.dram_tensor("partialT", [D, S], F32, kind="ExternalOutput")

    # ---- internal DRAM ----
    cq_shard = nc.dram_tensor("cq_shard", [QL, TOK], F16)
    cq_gat = nc.dram_tensor("cq_gat", [NCORE, QL, TOK], F16,
                            addr_space="Shared")
    kv_shard = nc.dram_tensor("kv_shard", [KR, TOK], F16)
    kv_gat = nc.dram_tensor("kv_gat", [NCORE, KR, TOK], F16,
                            addr_space="Shared")
    qlat_d = nc.dram_tensor("qlat_d", [HC * KVL, S], F16)
    qropeT_d = nc.dram_tensor("qropeT_d", [HC * DR, S], F16)

    RG = [list(range(NCORE))]

    with tile.TileContext(nc) as tc, ExitStack() as rep_ctx:
        if repeat > 1:
            rep_ctx.enter_context(tc.For_i(0, repeat, 1))

        # Persistent SBUF residents (allocated up-front so their address
        # range never overlaps the phase-A/B pools - overlapping reuse
        # made every resident load WAR-wait on shard-phase matmul reads)
        resid = ExitStack()
        resA = resid.enter_context(tc.tile_pool(name="resA", bufs=1))
        ckvT_sb = []
        for m in range(KC):
            ckvT_sb.append(
                resA.tile([128, S], F16, tag=f"ckvT{m}", name=f"ckvTp{m}")
            )
        # k_rope duplicated across both 64-row halves so paired score
        # tiles' rope matmuls run concurrently in disjoint PE row groups
        kropeT_sb = resA.tile([128, S], F16, tag="kropeT", name="kropeTp")
        resC = resid.enter_context(tc.tile_pool(name="resC", bufs=1))
        ckv_sb = []
        for tci in range(NT):
            ckv_sb.append(
                resC.tile([128, KVL], F16, tag=f"ckv{tci}", name=f"ckv{tci}")
            )
        resO = resid.enter_context(tc.tile_pool(name="resO", bufs=1))
        outhT_sb = []
        for kc in range(HC * DV // 128):
            outhT_sb.append(
                resO.tile([128, S], F16, tag=f"outh{kc}", name=f"outhp{kc}")
            )

        # ===== phase A/B : token-sharded c_q and kv + AllGathers =====
        with ExitStack() as cab:
            ht_pool = cab.enter_context(tc.tile_pool(name="hts", bufs=1))
            wkv_pool = cab.enter_context(tc.tile_pool(name="wkvb", bufs=1))
            wqd_pool = cab.enter_context(tc.tile_pool(name="wqda", bufs=4))
            idp = cab.enter_context(tc.tile_pool(name="ida", bufs=1))
            identa = idp.tile([128, 128], F16, name="identA")
            nc.sync.dma_start(identa[:], identD[:, :])
            rkp = cab.enter_context(tc.tile_pool(name="rkp", bufs=1))
            ropek_sb = rkp.tile([DR, TOK], F32, name="ropeks_sb")
            nc.sync.dma_start(ropek_sb[:], ropeks[:, :])

            hts_sb = []
            wkv_sb = []

            psA_ctx = ExitStack()
            psA = psA_ctx.enter_context(
                tc.tile_pool(name="psA", bufs=1, space="PSUM")
            )
            ps = [psA.tile([128, 512], F32, tag=f"pa{tci}_{j}",
                           name=f"pa{tci}_{j}")
                  for tci in range(TC) for j in range(J3)]
            # ---- c_q shard (token-major, wqd + hTs streamed) ----
            for k in range(DC):
                t = ht_pool.tile([128, TOK], F16, tag=f"ht{k}",
                                 name=f"hts{k}")
                nc.sync.dma_start(t[:], hTs[k * 128:(k + 1) * 128, :])
                hts_sb.append(t)
                w = wqd_pool.tile([128, QL], F16, tag="wqd")
                nc.sync.dma_start(w[:], wqd[k * 128:(k + 1) * 128, :])
                for tci in range(TC):
                    for j in range(J3):
                        nc.tensor.matmul(
                            ps[tci * J3 + j][:],
                            hts_sb[k][:, tci * 128:(tci + 1) * 128],
                            w[:, j * 512:(j + 1) * 512],
                            start=(k == 0),
                            stop=(k == DC - 1),
                        )

            for k in range(DC):
                v = wkv_pool.tile([128, KR], F16, tag=f"wkv{k}",
                                  name=f"wkvb{k}")
                nc.sync.dma_start(v[:], wkvr[k * 128:(k + 1) * 128, :])
                wkv_sb.append(v)

            # transpose c_q to feature-major, store shard, gather
            with ExitStack() as cta:
                psTA = cta.enter_context(
                    tc.tile_pool(name="psTA", bufs=2, space="PSUM")
                )
                tmA = cta.enter_context(tc.tile_pool(name="tmA", bufs=2))
                cqT_pool = cta.enter_context(
                    tc.tile_pool(name="cqT", bufs=1)
                )
                cqT_sb = []
                for j in range(QLC):
                    cqT_sb.append(
                        cqT_pool.tile([128, TOK], F16, tag=f"cqT{j}",
                                      name=f"cqT{j}")
                    )
                for tci in range(TC):
                    for j in range(J3):
                        tm = tmA.tile([128, 512], F16, tag="tm")
                        nc.vector.tensor_copy(tm[:], ps[tci * J3 + j][:])
                        for i in range(4):
                            pt = psTA.tile([128, 128], F16, tag="ptA",
                                           name="ptA")
                            nc.tensor.matmul(
                                pt[:],
                                tm[:, i * 128:(i + 1) * 128],
                                identa[:],
                                is_transpose=True,
                            )
                            nc.vector.tensor_copy(
                                cqT_sb[j * 4 + i][:,
                                    tci * 128:(tci + 1) * 128],
                                pt[:],
                            )
                for j in range(QLC):
                    nc.gpsimd.dma_start(
                        cq_shard[j * 128:(j + 1) * 128, :], cqT_sb[j][:]
                    )
            psA_ctx.close()
            nc.gpsimd.collective_compute(
                "AllGather",
                mybir.AluOpType.bypass,
                replica_groups=RG,
                ins=[cq_shard[:, :]],
                outs=[cq_gat[:, :, :]],
            )

            # ---- kv shard (feature-major) ----
            with ExitStack() as ckv_ctx:
                psB = ckv_ctx.enter_context(
                    tc.tile_pool(name="psB", bufs=2, space="PSUM")
                )
                psRB = ckv_ctx.enter_context(
                    tc.tile_pool(name="psRB", bufs=1, space="PSUM")
                )
                evB = ckv_ctx.enter_context(tc.tile_pool(name="evB", bufs=2))
                for m in range(KC):
                    pk = psB.tile([128, TOK], F32, tag="pkB")
                    for k in range(DC):
                        nc.tensor.matmul(
                            pk[:],
                            wkv_sb[k][:, m * 128:(m + 1) * 128],
                            hts_sb[k][:],
                            start=(k == 0),
                            stop=(k == DC - 1),
                        )
                    ev = evB.tile([128, TOK], F16, tag="evB")
                    nc.vector.tensor_copy(ev[:], pk[:])
                    nc.gpsimd.dma_start(
                        kv_shard[m * 128:(m + 1) * 128, :], ev[:]
                    )
                # k_rope shard + rope apply
                pr = psRB.tile([DR, TOK], F32, tag="prB")
                for k in range(DC):
                    nc.tensor.matmul(
                        pr[:],
                        wkv_sb[k][:, KVL:KVL + DR],
                        hts_sb[k][:],
                        start=(k == 0),
                        stop=(k == DC - 1),
                    )
                c_ = ropek_sb[0:HDR, :]
                s_ = ropek_sb[HDR:DR, :]
                kev = evB.tile([DR, TOK], F16, tag="kevB")
                t1 = evB.tile([HDR, TOK], F32, tag="t1B")
                t2 = evB.tile([HDR, TOK], F32, tag="t2B")
                nc.vector.tensor_mul(t1[:], pr[0:HDR, :], c_)
                nc.vector.tensor_mul(t2[:], pr[HDR:DR, :], s_)
                nc.vector.tensor_sub(kev[0:HDR, :], t1[:], t2[:])
                t3 = evB.tile([HDR, TOK], F32, tag="t3B")
                t4 = evB.tile([HDR, TOK], F32, tag="t4B")
                nc.vector.tensor_mul(t3[:], pr[0:HDR, :], s_)
                nc.vector.tensor_mul(t4[:], pr[HDR:DR, :], c_)
                nc.vector.tensor_add(kev[HDR:DR, :], t3[:], t4[:])
                nc.gpsimd.dma_start(kv_shard[KVL:KVL + DR, :], kev[:])
            nc.gpsimd.collective_compute(
                "AllGather",
                mybir.AluOpType.bypass,
                replica_groups=RG,
                ins=[kv_shard[:, :]],
                outs=[kv_gat[:, :, :]],
            )

        # ====== phases C/D/E ======
        with ExitStack() as cshared:

            # ============ phase C : q-up from gathered c_q ============
            cc = cshared.enter_context(ExitStack())
            wqu_pool = cc.enter_context(tc.tile_pool(name="wquc", bufs=1))
            wqu_sb = []
            for k in range(QLC):
                u = wqu_pool.tile([128, QH], F16, tag=f"wqu{k}",
                                  name=f"wquc{k}")
                nc.sync.dma_start(u[:], wqu[k * 128:(k + 1) * 128, :])
                wqu_sb.append(u)
            wuk_pool = cc.enter_context(tc.tile_pool(name="wukc", bufs=1))
            wuk_sb = []
            for m in range(HC):
                t = wuk_pool.tile([128, KVL], F16, tag=f"wuk{m}",
                                  name=f"wukc{m}")
                nc.sync.dma_start(t[:], wuk[m * 128:(m + 1) * 128, :])
                wuk_sb.append(t)
            ropeq_pool = cc.enter_context(tc.tile_pool(name="rpq", bufs=1))
            ropeq_sb = ropeq_pool.tile([DR, S], F32, name="ropeq_sb")
            nc.sync.dma_start(ropeq_sb[:], ropeq[:, :])

            cq_pool = cc.enter_context(tc.tile_pool(name="cqg", bufs=2))
            psq = cc.enter_context(
                tc.tile_pool(name="psq", bufs=1, space="PSUM")
            )
            psr = cc.enter_context(
                tc.tile_pool(name="psr", bufs=1, space="PSUM")
            )
            pslat = cc.enter_context(
                tc.tile_pool(name="pslat", bufs=2, space="PSUM")
            )
            qn_pool = cc.enter_context(tc.tile_pool(name="qn", bufs=2))
            qlat_ev = cc.enter_context(tc.tile_pool(name="qlev", bufs=2))
            rtmp = cc.enter_context(tc.tile_pool(name="rtmp", bufs=1))
            qr_ev = cc.enter_context(tc.tile_pool(name="qrev", bufs=2))

            BPG = 512 // TOK  # gathered blocks per 512-token group

            RC = (HC * DR + 127) // 128
            HK = QLC // 2
            for ng in range(SG):
                # contraction split in two halves so the first half of the
                # pipelined c_q gather is consumed while the second flies
                cqg = []
                pqs = [psq.tile([128, 512], F32, tag=f"pq{m}",
                                name=f"pq{m}")
                       for m in range(HC)]
                prs = [psr.tile([128, 512], F32, tag=f"pr{rc}",
                                name=f"pr{rc}")
                       for rc in range(RC)]
                for kh in range(2):
                    for k in range(kh * HK, (kh + 1) * HK):
                        t = cq_pool.tile([128, 512], F16, tag=f"cq{k}",
                                         name=f"cqg{k}")
                        for half in range(BPG):
                            nc.sync.dma_start(
                                t[:, half * TOK:(half + 1) * TOK],
                                cq_gat[ng * BPG + half,
                                       k * 128:(k + 1) * 128, :],
                            )
                        cqg.append(t)
                    for m in range(HC):
                        for k in range(kh * HK, (kh + 1) * HK):
                            nc.tensor.matmul(
                                pqs[m][:],
                                wqu_sb[k][:, m * 128:(m + 1) * 128],
                                cqg[k][:],
                                start=(k == 0),
                                stop=(k == QLC - 1),
                            )
                    for rc in range(RC):
                        cbase = HC * DN + rc * 128
                        for k in range(kh * HK, (kh + 1) * HK):
                            nc.tensor.matmul(
                                prs[rc][:],
                                wqu_sb[k][:, cbase:cbase + 128],
                                cqg[k][:],
                                start=(k == 0),
                                stop=(k == QLC - 1),
                            )
                # nope heads -> q_lat
                for m in range(HC):
                    qn = qn_pool.tile([128, 512], F16, tag="qn")
                    nc.vector.tensor_copy(qn[:], pqs[m][:])
                    for kc in range(KC):
                        pl = pslat.tile([128, 512], F32, tag="pl")
                        nc.tensor.matmul(
                            pl[:],
                            wuk_sb[m][:, kc * 128:(kc + 1) * 128],
                            qn[:],
                            start=True,
                            stop=True,
                        )
                        ev = qlat_ev.tile([128, 512], F16, tag="qlev")
                        nc.vector.tensor_scalar_mul(ev[:], pl[:], cfg.scale)
                        nc.gpsimd.dma_start(
                            qlat_d[m * KVL + kc * 128:
                                   m * KVL + (kc + 1) * 128,
                                   ng * 512:(ng + 1) * 512],
                            ev[:],
                        )
                # rope heads (q) - packed 128-col chunks (2 heads/chunk)
                for rc in range(RC):
                    rows = 128
                    pr = prs[rc]
                    c_ = ropeq_sb[0:HDR, ng * 512:(ng + 1) * 512]
                    s_ = ropeq_sb[HDR:DR, ng * 512:(ng + 1) * 512]
                    qr = qr_ev.tile([rows, 512], F16, tag="qrev")
                    for hh in range(rows // DR):
                        o = hh * DR
                        t1 = rtmp.tile([32, 512], F32, tag="t1")
                        t2 = rtmp.tile([32, 512], F32, tag="t2")
                        nc.vector.tensor_mul(t1[:], pr[o:o + HDR, :], c_)
                        nc.vector.tensor_mul(t2[:], pr[o + HDR:o + DR, :], s_)
                        nc.vector.tensor_sub(qr[o:o + HDR, :], t1[:], t2[:])
                        t3 = rtmp.tile([32, 512], F32, tag="t3")
                        t4 = rtmp.tile([32, 512], F32, tag="t4")
                        nc.vector.tensor_mul(t3[:], pr[o:o + HDR, :], s_)
                        nc.vector.tensor_mul(t4[:], pr[o + HDR:o + DR, :], c_)
                        nc.vector.tensor_add(qr[o + HDR:o + DR, :], t3[:], t4[:])
                    nc.gpsimd.dma_start(
                        qropeT_d[rc * 128:rc * 128 + rows,
                                 ng * 512:(ng + 1) * 512],
                        qr[:],
                    )
            cc.close()

            # ====== phase D : residents + token-major ckv ======
            for c in range(NCORE):
                for m in range(KC):
                    nc.gpsimd.dma_start(
                        ckvT_sb[m][:, c * TOK:(c + 1) * TOK],
                        kv_gat[c, m * 128:(m + 1) * 128, :],
                    )
                for hrow in range(2):
                    nc.gpsimd.dma_start(
                        kropeT_sb[hrow * DR:(hrow + 1) * DR,
                                  c * TOK:(c + 1) * TOK],
                        kv_gat[c, KVL:KVL + DR, :],
                    )
            with ExitStack() as cd:
                idp = cd.enter_context(tc.tile_pool(name="id1", bufs=1))
                ident = idp.tile([128, 128], F16, name="ident")
                nc.sync.dma_start(ident[:], identD[:, :])
                pst = cd.enter_context(
                    tc.tile_pool(name="pst", bufs=3, space="PSUM")
                )
                for tci in range(NT):
                    for m in range(KC):
                        pt = pst.tile([128, 128], F16, tag="pt", name="pt")
                        nc.tensor.matmul(
                            pt[:],
                            ckvT_sb[m][:, tci * 128:(tci + 1) * 128],
                            ident[:],
                            is_transpose=True,
                        )
                        nc.vector.tensor_copy(
                            ckv_sb[tci][:, m * 128:(m + 1) * 128], pt[:]
                        )


            # ========= phase E : attention (transposed scores) =========
            ce = cshared.enter_context(ExitStack())
            res = ce.enter_context(tc.tile_pool(name="res2", bufs=1))
            mask_sb = None
            ones_sb = None
            wuvT_sb = []

            def _load_res2():
                nonlocal mask_sb, ones_sb
                mask_sb = res.tile([128, 128], F32, tag="mask",
                                   name="mask_sb")
                nc.sync.dma_start(mask_sb[:], trimask[:, :])
                ones_sb = res.tile([128, 1], F16, tag="ones", name="ones_sb")
                nc.vector.memset(ones_sb[:], 1.0)
                for kc in range(KC):
                    t = res.tile([128, HC * DV], F16, tag=f"wuvT{kc}",
                                 name=f"wuvT{kc}")
                    nc.sync.dma_start(
                        t[:], wuvT[kc * 128:(kc + 1) * 128, :]
                    )
                    wuvT_sb.append(t)

            qr_pool = ce.enter_context(tc.tile_pool(name="qrh", bufs=3))
            qlat_pool = ce.enter_context(tc.tile_pool(name="qlat", bufs=3))
            pb_pool = ce.enter_context(tc.tile_pool(name="pb", bufs=4))
            olT_pool = ce.enter_context(tc.tile_pool(name="olT", bufs=1))
            stat_pool = ce.enter_context(tc.tile_pool(name="stat", bufs=2))

            psc = ce.enter_context(
                tc.tile_pool(name="psc", bufs=2, space="PSUM")
            )
            pso = ce.enter_context(
                tc.tile_pool(name="pso", bufs=1, space="PSUM")
            )
            psd = ce.enter_context(
                tc.tile_pool(name="psd", bufs=1, space="PSUM")
            )
            psh = ce.enter_context(
                tc.tile_pool(name="psh", bufs=1, space="PSUM")
            )

            wo_pool = ce.enter_context(tc.tile_pool(name="wo", bufs=2))
            oev = ce.enter_context(tc.tile_pool(name="oev", bufs=2))
            HDVC = HC * DV // 128

            for qg in range(SG):
                for h in range(HC):
                    qr_h = qr_pool.tile([128, 512], F16, tag="qrh")
                    for hrow in range(2):
                        nc.sync.dma_start(
                            qr_h[hrow * DR:(hrow + 1) * DR, :],
                            qropeT_d[h * DR:(h + 1) * DR,
                                     qg * 512:(qg + 1) * 512],
                        )
                    qlat_t = []
                    for kc in range(KC):
                        t = qlat_pool.tile([128, 512], F16, tag=f"qlat{kc}",
                                           name=f"qlat{kc}")
                        nc.sync.dma_start(
                            t[:],
                            qlat_d[h * KVL + kc * 128:
                                   h * KVL + (kc + 1) * 128,
                                   qg * 512:(qg + 1) * 512],
                        )
                        qlat_t.append(t)
                    if h == 0 and qg == 0:
                        _load_res2()
                    olT_ps = [
                        pso.tile([128, 512], F32, tag=f"olT{kc}",
                                 name=f"olTps{kc}")
                        for kc in range(KC)
                    ]
                    d_ps = psd.tile([1, 512], F32, tag="dps", name="dps")
                    NJ = 4 * (qg + 1)
                    for jp in range(0, NJ, 2):
                        pair = (jp, jp + 1)
                        cos = []
                        scs = []
                        for pi, j in enumerate(pair):
                            kdiag = j - 4 * qg
                            co = kdiag * 128 if kdiag >= 0 else 0
                            cos.append(co)
                            sc = psc.tile([128, 512], F32, tag="sc",
                                          name="scp")
                            scs.append(sc)
                            for kc in range(KC):
                                nc.tensor.matmul(
                                    sc[:, co:512],
                                    ckvT_sb[kc][:, j * 128:(j + 1) * 128],
                                    qlat_t[kc][:, co:512],
                                    start=(kc == 0),
                                    stop=False,
                                )
                        # the two rope matmuls use disjoint 64-row groups
                        # and run concurrently on the PE
                        for pi, j in enumerate(pair):
                            nc.tensor.matmul(
                                scs[pi][:, cos[pi]:512],
                                kropeT_sb[pi * DR:(pi + 1) * DR,
                                          j * 128:(j + 1) * 128],
                                qr_h[pi * DR:(pi + 1) * DR, cos[pi]:512],
                                start=False,
                                stop=True,
                            )
                        for pi, j in enumerate(pair):
                            kdiag = j - 4 * qg
                            co = cos[pi]
                            sc = scs[pi]
                            if kdiag >= 0:
                                nc.vector.tensor_add(
                                    sc[:, co:co + 128],
                                    sc[:, co:co + 128],
                                    mask_sb[:],
                                )
                            pb = pb_pool.tile([128, 512], F16, tag="pb")
                            nc.scalar.activation(
                                pb[:, co:512], sc[:, co:512], EXP, bias=0.0
                            )
                            for kc in range(KC):
                                nc.tensor.matmul(
                                    olT_ps[kc][:, co:512],
                                    ckv_sb[j][:, kc * 128:(kc + 1) * 128],
                                    pb[:, co:512],
                                    start=(j == 0),
                                    stop=(j == NJ - 1),
                                )
                            nc.tensor.matmul(
                                d_ps[:, co:512],
                                ones_sb[:],
                                pb[:, co:512],
                                start=(j == 0),
                                stop=(j == NJ - 1),
                            )
                    # denominators -> 1/d as exp(-ln d) on the scalar
                    # engine ([1,512] on the DVE is lane-serial ~3.3us and
                    # blocks the DVE queue; ACT does it in ~0.9us)
                    lnd = stat_pool.tile([1, 512], F32, tag="lnd")
                    nc.scalar.activation(
                        lnd[:], d_ps[:], mybir.ActivationFunctionType.Ln
                    )
                    rinv = stat_pool.tile([1, 512], F32, tag="rinv")
                    nc.scalar.activation(rinv[:], lnd[:], EXP, scale=-1.0)
                    rbc = stat_pool.tile([128, 512], F32, tag="rbc")
                    nc.gpsimd.partition_broadcast(rbc[:], rinv[:])
                    # evacuate out_lat^T, expand with wuv, normalize
                    olT_sb = []
                    for kc in range(KC):
                        t = olT_pool.tile([128, 512], F16, tag=f"olTs{kc}",
                                          name=f"olTs{kc}")
                        nc.vector.tensor_copy(t[:], olT_ps[kc][:])
                        olT_sb.append(t)
                    poh = psh.tile([128, 512], F32, tag="poh", name="poh")
                    for kc in range(KC):
                        nc.tensor.matmul(
                            poh[:],
                            wuvT_sb[kc][:, h * DV:(h + 1) * DV],
                            olT_sb[kc][:],
                            start=(kc == 0),
                            stop=(kc == KC - 1),
                        )
                    nc.vector.tensor_mul(
                        outhT_sb[h][:, qg * 512:(qg + 1) * 512],
                        poh[:],
                        rbc[:],
                    )
                # ---- final projection, two passes: bulk after the
                # second-to-last group (its dense matmuls fill the latency
                # bubbles of the thin causal groups processed last),
                # remainder at the very end
                if qg == SG - 2:
                    proj = list(range(SG - 1))
                elif qg == SG - 1:
                    proj = [SG - 1]
                else:
                    proj = []
                if proj:
                    for mg in range(DC // 4):
                        wo = []
                        for kc in range(HDVC):
                            t = wo_pool.tile([128, 512], F16, tag=f"wo{kc}",
                                             name=f"wo{kc}")
                            nc.sync.dma_start(
                                t[:],
                                wout[kc * 128:(kc + 1) * 128,
                                     mg * 512:(mg + 1) * 512],
                            )
                            wo.append(t)
                        for ml in range(4):
                            m = mg * 4 + ml
                            for g in proj:
                                pf = psc.tile([128, 512], F32, tag="sc",
                                              name="pf3")
                                for kc in range(HDVC):
                                    nc.tensor.matmul(
                                        pf[:],
                                        wo[kc][:, ml * 128:(ml + 1) * 128],
                                        outhT_sb[kc][:,
                                            g * 512:(g + 1) * 512],
                                        start=(kc == 0),
                                        stop=(kc == HDVC - 1),
                                    )
                                ev = oev.tile([128, 512], F32, tag="oev")
                                nc.any.tensor_copy(ev[:], pf[:])
                                nc.gpsimd.dma_start(
                                    partialT[m * 128:(m + 1) * 128,
                                             g * 512:(g + 1) * 512],
                                    ev[:],
                                )

        resid.close()

    nc.compile()
    return nc


# ---------------- host-side prep ----------------

def _yarn_tables(cfg: Cfg):
    """cos/sin tables [HDR, S], matching the reference YaRN rope."""
    freqs = 1.0 / BASE ** (
        np.arange(0, cfg.DR, 2, dtype=np.float32) / np.float32(cfg.DR)
    )
    wavelengths = 2.0 * np.pi / freqs
    ramp = np.clip(
        (wavelengths / OLD_CTX - BSLOW) / (BFAST - BSLOW), 0.0, 1.0
    ).astype(np.float32)
    scale = 1.0 - ramp + ramp * FACTOR
    inv_freq = (freqs / scale).astype(np.float32)
    pos = np.arange(cfg.S, dtype=np.float32)
    f = pos[:, None] * inv_freq[None, :]  # [S, HDR]
    cos = (np.cos(f) * MSCALE).astype(np.float32).T.copy()  # [HDR, S]
    sin = (np.sin(f) * MSCALE).astype(np.float32).T.copy()
    return cos, sin


def _trimask():
    # scores^T diag-tile mask on the first 128 computed q-columns:
    # m[t, c] = 0 where c >= t, else -inf
    t = np.arange(128)[:, None]
    c = np.arange(128)[None, :]
    return np.where(c >= t, 0.0, MASK_NEG).astype(np.float32)


def make_in_maps(cfg: Cfg, inputs: dict) -> list[dict]:
    f16 = np.float16
    hidden = np.asarray(inputs["hidden_states"], dtype=np.float32)
    w_q_down = np.asarray(inputs["w_q_down"], dtype=f16)
    w_q_up_nope = np.asarray(inputs["w_q_up_nope"], dtype=f16)
    w_q_up_rope = np.asarray(inputs["w_q_up_rope"], dtype=f16)
    w_kv_down = np.asarray(inputs["w_kv_down"], dtype=f16)
    w_k_rope = np.asarray(inputs["w_k_rope"], dtype=f16)
    w_uk = np.asarray(inputs["w_uk"], dtype=f16)
    w_uv = np.asarray(inputs["w_uv"], dtype=f16)
    w_out = np.asarray(inputs["w_out"], dtype=f16)

    HC, DN, DR, DV, KVL = cfg.HC, cfg.DN, cfg.DR, cfg.DV, cfg.KVL
    hT = np.ascontiguousarray(hidden[0].T.astype(f16))  # [D, S]
    wkvr = np.ascontiguousarray(
        np.concatenate([w_kv_down, w_k_rope], axis=1)
    )  # [D, KVL+DR]
    cos, sin = _yarn_tables(cfg)
    sc = np.float32(cfg.scale)
    ropeq = np.ascontiguousarray(
        np.concatenate([cos * sc, sin * sc], axis=0)
    )  # [DR, S] (scaled for q)
    ropek = np.ascontiguousarray(np.concatenate([cos, sin], axis=0))
    trimask = _trimask()
    identD = np.eye(128, dtype=f16)

    wuv3 = w_uv.reshape(cfg.H, DV, KVL)
    in_maps = []
    for c in range(cfg.n_cores):
        wqu_c = np.ascontiguousarray(
            np.concatenate(
                [
                    w_q_up_nope[:, c * HC * DN:(c + 1) * HC * DN],
                    w_q_up_rope[:, c * HC * DR:(c + 1) * HC * DR],
                ],
                axis=1,
            )
        )  # [QL, QH]
        wuk_c = np.ascontiguousarray(
            w_uk[c * HC * DN:(c + 1) * HC * DN, :]
        )  # [HC*DN, KVL]
        wuvT_c = np.ascontiguousarray(
            np.concatenate(
                [wuv3[h].T for h in range(c * HC, (c + 1) * HC)], axis=1
            )
        )  # [KVL, HC*DV]
        wout_c = np.ascontiguousarray(
            w_out[c * HC * DV:(c + 1) * HC * DV, :]
        )  # [HC*DV, D]
        in_maps.append(
            {
                "hTs": np.ascontiguousarray(
                    hT[:, c * cfg.TOK:(c + 1) * cfg.TOK]
                ),
                "wqd": w_q_down,
                "wqu": wqu_c,
                "wkvr": wkvr,
                "wuk": wuk_c,
                "wuvT": wuvT_c,
                "wout": wout_c,
                "ropeq": ropeq,
                "ropeks": np.ascontiguousarray(
                    ropek[:, c * cfg.TOK:(c + 1) * cfg.TOK]
                ),
                "trimask": trimask,
                "identD": identD,
            }
        )
    return in_maps


_NC_CACHE: dict = {}
LAST_T: dict = {}


def _get_nc(cfg: Cfg):
    if cfg not in _NC_CACHE:
        _NC_CACHE[cfg] = build_bass(cfg)
    return _NC_CACHE[cfg]


def run(cfg: Cfg, inputs: dict):
    import time as _time

    t0 = _time.time()
    nc = _get_nc(cfg)
    t1 = _time.time()
    in_maps = make_in_maps(cfg, inputs)
    t2 = _time.time()
    res = run_bass_kernel_spmd(nc, in_maps, list(range(cfg.n_cores)))
    t3 = _time.time()
    parts = [r["partialT"] for r in res.results]
    acc = parts[0].astype(np.float32)
    for p in parts[1:]:
        acc = acc + p
    out = np.ascontiguousarray(acc.T)[None]  # [1, S, D]
    t4 = _time.time()
    LAST_T.update(
        build=t1 - t0, prep=t2 - t1, spmd=t3 - t2, gather=t4 - t3
    )
    return out


def kernel(**inputs) -> np.ndarray:
    cfg = Cfg()
    return run(cfg, inputs)


if __name__ == "__main__":
    cfg = Cfg()
    nc = build_bass(cfg)
    print("built + compiled ok")
